# revision 33
# baseline (speedup 1.0000x reference)
"""Margin-based triplet criterion (loss_fn) on 8 TRN2 NeuronCores.

Strategy (anchor-block sharding; each core owns B/8 = 512 anchor rows):
  - Host: quantize batch*0.5 to fp8 e4m3 (so the PE Gram values G/4 stay in
    e4m3 range for the fp8 store), build bT [4, 128, 4096] (D-major
    transpose, K-chunked, replicated) and aT [128, 4, 512] = the core's
    anchor-block columns (stationary). Re-sort triplets to the core owning
    their anchor (loss is a permutation-invariant sum), pad with masked
    dummies (bm=+LARGE, bp=-LARGE => zero contribution). Precompute
    per-triplet ssum = s[ia]+s[ip|n] (f32, from the dequantized rows),
    hinge offsets bm/bp, and flat gather indices into the Gram block
    laid out [p, m, col] (anchor a -> partition a%128, sub-row a//128).
  - Device: PE computes the Gram block (512 anchors x 4096) with fp8
    DoubleRow matmuls (K=256 per pass, N=512 per PSUM bank; 64 matmuls).
    DVE/ACT/Pool alternate downconverting PSUM f32 -> fp8 SBUF staging
    tiles [128, 4096] which are DMA'd (4 stores) to a DRAM scratch tile.
    One indirect (SWDGE) gather pulls the 2*NCOLS per-partition dot
    products as scalars. Epilogue: d^2 = ssum - 8*g (clamped), d =
    sqrt(d^2 + eps) on ACT, hinges pos = relu(d_ap - bm), neg =
    relu(bp - d_an), z = pos + neg, indicator z > 0, free-dim reductions
    -> [128, 2] (sum, count) per core.
  - Host: sum the 8x128 partials, loss = total / max(count, 1) if count>0.
"""

import numpy as np
import ml_dtypes
from contextlib import ExitStack

import concourse.bass as bass
import concourse.bacc as bacc
import concourse.tile as tile
from concourse import mybir
from concourse.bass_utils import run_bass_kernel_spmd

N_CORES = 8
B, D, T, C = 4096, 512, 65536, 100
A_LOC = B // N_CORES            # 512 anchors per core
KCH = 4                         # K chunks of 128 (D = 512)
M_TILES = A_LOC // 128          # 4 anchor tiles per core
NBANK = 512                     # matmul N per PSUM bank (f32)
HALF = 1024                     # columns per copy unit (2 banks)
NCOLS = 68                      # padded triplet columns per partition
NC2 = 2 * NCOLS                 # ap|an concatenated
MARGIN = 0.2
EPS = 1e-8
LARGE = 4e6

f32 = mybir.dt.float32
bf16 = mybir.dt.bfloat16
fp8 = mybir.dt.float8e4
i32 = mybir.dt.int32
FP8NP = ml_dtypes.float8_e4m3

_CACHE = {}


def _build_nc():
    nc = bacc.Bacc(
        "TRN2", target_bir_lowering=False, debug=False,
        enable_asserts=False, num_devices=N_CORES,
    )
    bT = nc.dram_tensor("bT", [KCH, 128, B], fp8, kind="ExternalInput")
    aT = nc.dram_tensor("aT", [128, KCH, A_LOC], fp8, kind="ExternalInput")
    gidx = nc.dram_tensor("gidx", [128, NC2], i32, kind="ExternalInput")
    ssum = nc.dram_tensor("ssum", [128, NC2], f32, kind="ExternalInput")
    bmp = nc.dram_tensor("bmp", [128, NC2], f32, kind="ExternalInput")
    outp = nc.dram_tensor("out", [128, 2], f32, kind="ExternalOutput")

    with tile.TileContext(nc) as tc, ExitStack() as ctx:
        const_pool = ctx.enter_context(tc.tile_pool(name="const", bufs=1))
        psum_pool = ctx.enter_context(tc.tile_pool(name="ps", bufs=3, space="PSUM"))
        gsb_pool = ctx.enter_context(tc.tile_pool(name="gsb", bufs=6))
        gdram_pool = ctx.enter_context(tc.tile_pool(name="gdram", bufs=1, space="DRAM"))
        epi_pool = ctx.enter_context(tc.tile_pool(name="epi", bufs=1))

        # Loads ordered so PE can start ASAP: stationary aT, then all 4 k
        # chunks of the first column half (enough for the h=0 units), then
        # the second half, then gather/epilogue operands.
        aT_sb = const_pool.tile([128, KCH, A_LOC], fp8)
        bT_sb = const_pool.tile([128, KCH, B], fp8)
        c0 = slice(0, B // 2)
        c1 = slice(B // 2, B)
        nc.sync.dma_start(aT_sb[:, 0:2, :], aT[:, 0:2, :])
        nc.sync.dma_start(bT_sb[:, 0, c0], bT[0, :, c0])
        nc.sync.dma_start(bT_sb[:, 1, c0], bT[1, :, c0])
        nc.sync.dma_start(aT_sb[:, 2:4, :], aT[:, 2:4, :])
        nc.sync.dma_start(bT_sb[:, 2, c0], bT[2, :, c0])
        nc.sync.dma_start(bT_sb[:, 3, c0], bT[3, :, c0])
        for k in range(KCH):
            nc.sync.dma_start(bT_sb[:, k, c1], bT[k, :, c1])

        eps_sb = const_pool.tile([128, 1], f32)
        nc.vector.memset(eps_sb[:], EPS)
        idx_sb = const_pool.tile([128, NC2], i32)
        nc.sync.dma_start(idx_sb[:], gidx[:])
        ssum_sb = const_pool.tile([128, NC2], f32)
        nc.sync.dma_start(ssum_sb[:], ssum[:])
        bmp_sb = const_pool.tile([128, NC2], f32)
        nc.sync.dma_start(bmp_sb[:], bmp[:])

        # Gram scratch, laid out [p, m, col]: anchor a = 128m + p
        g_t = gdram_pool.tile([128, M_TILES * B], fp8)

        # PE warmup: dummy matmuls gated only on the (early) aT load keep the
        # PE continuously busy until the first bT chunks land, so the real
        # matmuls issue against a ramped-up (full-rate) tensor engine.
        warm = psum_pool.tile([128, HALF // NBANK, NBANK], f32, tag="ps")
        for _ in range(6):
            nc.tensor.matmul(
                warm[:, 0, :],
                lhsT=aT_sb[:, 0:2, 0:128],
                rhs=aT_sb[:, 0:2, 0:NBANK],
                start=True, stop=True,
                perf_mode=mybir.MatmulPerfMode.DoubleRow,
            )
        for _ in range(3):
            nc.tensor.matmul(
                warm[:, 1, :],
                lhsT=aT_sb[:, 2:4, 0:128],
                rhs=aT_sb[:, 2:4, 0:NBANK],
                start=True, stop=True,
                perf_mode=mybir.MatmulPerfMode.DoubleRow,
            )

        # Gram block: units (column half h, anchor tile m); h-outer so the
        # first 4 units only need the first half of the bT columns.
        # Both copy engines split each unit (halves PSUM release latency).
        for h in range(B // HALF):
            for m in range(M_TILES):
                ps = psum_pool.tile([128, HALF // NBANK, NBANK], f32, tag="ps")
                # k2-outer so PE only waits on bT chunks 2*k2, 2*k2+1
                for k2 in range(KCH // 2):
                    for b in range(HALF // NBANK):
                        nc.tensor.matmul(
                            ps[:, b, :],
                            lhsT=aT_sb[:, 2 * k2:2 * k2 + 2,
                                       m * 128:(m + 1) * 128],
                            rhs=bT_sb[:, 2 * k2:2 * k2 + 2,
                                      h * HALF + b * NBANK:
                                      h * HALF + (b + 1) * NBANK],
                            start=(k2 == 0), stop=(k2 == KCH // 2 - 1),
                            perf_mode=mybir.MatmulPerfMode.DoubleRow,
                        )
                gsb = gsb_pool.tile([128, HALF], fp8, tag="gsb")
                if (h * M_TILES + m) % 2 == 1:
                    nc.scalar.copy(out=gsb[:], in_=ps[:, :, :])
                else:
                    nc.vector.tensor_copy(out=gsb[:], in_=ps[:, :, :])
                nc.sync.dma_start(
                    g_t[:, m * B + h * HALF:m * B + (h + 1) * HALF], gsb[:])

        # Per-triplet dot products: scalar gather from the DRAM Gram block
        g_vals = epi_pool.tile([128, NC2], fp8, tag="gv")
        nc.gpsimd.indirect_dma_start(
            out=g_vals[:],
            out_offset=None,
            in_=g_t[:],
            in_offset=bass.IndirectOffsetOnAxis(ap=idx_sb[:], axis=1),
        )

        # d^2 = ssum - 8*g = -8*(g - ssum/8); host passes ssum/8.
        dif = epi_pool.tile([128, NC2], f32, tag="dif")
        nc.vector.tensor_tensor(
            out=dif[:], in0=g_vals[:], in1=ssum_sb[:],
            op=mybir.AluOpType.subtract)
        dsq = epi_pool.tile([128, NC2], f32, tag="dsq")
        nc.vector.tensor_scalar(
            out=dsq[:], in0=dif[:], scalar1=-8.0, scalar2=0.0,
            op0=mybir.AluOpType.mult, op1=mybir.AluOpType.max)
        dist = epi_pool.tile([128, NC2], f32, tag="dist")
        nc.scalar.activation(
            out=dist[:], in_=dsq[:],
            func=mybir.ActivationFunctionType.Sqrt, bias=eps_sb[:])

        # hinges: pos = relu(d_ap - bm) (DVE), neg = relu(bp - d_an) (Pool)
        hing = epi_pool.tile([128, NC2], f32, tag="hing")
        nc.vector.tensor_tensor(
            out=hing[:, 0:NCOLS], in0=dist[:, 0:NCOLS], in1=bmp_sb[:, 0:NCOLS],
            op=mybir.AluOpType.subtract)
        nc.vector.tensor_tensor(
            out=hing[:, NCOLS:NC2], in0=bmp_sb[:, NCOLS:NC2],
            in1=dist[:, NCOLS:NC2], op=mybir.AluOpType.subtract)
        nc.vector.tensor_scalar_max(hing[:], hing[:], 0.0)
        z = epi_pool.tile([128, NCOLS], f32, tag="z")
        outsb = epi_pool.tile([128, 2], f32, tag="outsb")
        nc.vector.tensor_tensor(
            out=z[:], in0=hing[:, 0:NCOLS], in1=hing[:, NCOLS:NC2],
            op=mybir.AluOpType.add)
        ind = epi_pool.tile([128, NCOLS], f32, tag="ind")
        nc.vector.tensor_scalar(
            out=ind[:], in0=z[:], scalar1=0.0, scalar2=0.0,
            op0=mybir.AluOpType.is_gt, op1=mybir.AluOpType.add,
            accum_out=outsb[:, 1:2])
        nc.vector.tensor_reduce(
            out=outsb[:, 0:1], in_=z[:], axis=mybir.AxisListType.X,
            op=mybir.AluOpType.add)
        nc.sync.dma_start(outp[:], outsb[:])

    nc.compile()
    return nc


def _prep_inputs(batch, beta, labels, triplets):
    batch = np.asarray(batch, dtype=np.float32)
    beta = np.asarray(beta, dtype=np.float32)
    labels = np.asarray(labels).astype(np.int64)
    triplets = np.asarray(triplets).astype(np.int64)

    q = (0.5 * batch).astype(FP8NP)          # device rows (scaled by 1/2)
    qf = q.astype(np.float32)
    # effective embedding is 2*q; s = |2q|^2
    s = 4.0 * (qf.astype(np.float64) ** 2).sum(axis=1)
    s = s.astype(np.float32)

    # bT[k, d, j] = q[j, 128k + d]
    bT_all = np.ascontiguousarray(q.T.reshape(KCH, 128, B))

    ia, ip, iN = triplets[:, 0], triplets[:, 1], triplets[:, 2]
    core = ia // A_LOC
    al = ia % A_LOC
    b = beta[labels[ia]].astype(np.float32)

    # Gram scratch layout [p, m, col]: flat = (al%128)*(M_TILES*B) + (al//128)*B + col
    CAP = 128 * NCOLS
    in_maps = []
    for c in range(N_CORES):
        sel = np.nonzero(core == c)[0]
        n = len(sel)
        assert n <= CAP, f"core {c} overflow: {n} > {CAP}"

        def packi(vals, fill):
            arr = np.full(CAP, fill, dtype=np.int64)
            arr[:n] = vals
            return arr.reshape(NCOLS, 128).T.astype(np.int32)

        def packf(vals, fill):
            arr = np.full(CAP, fill, dtype=np.float32)
            arr[:n] = vals
            return arr.reshape(NCOLS, 128).T

        alc = al[sel]
        base = (alc % 128) * (M_TILES * B) + (alc // 128) * B
        gidx = np.concatenate(
            [packi(base + ip[sel], 0), packi(base + iN[sel], 0)], axis=1)
        # device computes d^2 = -8*(g - ssum/8): pass ssum/8
        ssum = np.concatenate(
            [packf((s[ia[sel]] + s[ip[sel]]) / 8.0, 0.0),
             packf((s[ia[sel]] + s[iN[sel]]) / 8.0, 0.0)], axis=1)
        # first half bm (pad +LARGE kills pos), second half bp (pad -LARGE
        # kills neg: device computes relu(bp - d))
        bmp = np.concatenate(
            [packf(b[sel] - MARGIN, LARGE), packf(b[sel] + MARGIN, -LARGE)],
            axis=1)

        aTc = np.ascontiguousarray(
            bT_all[:, :, c * A_LOC:(c + 1) * A_LOC].transpose(1, 0, 2))

        in_maps.append({
            "bT": bT_all,
            "aT": aTc,
            "gidx": np.ascontiguousarray(gidx),
            "ssum": np.ascontiguousarray(ssum),
            "bmp": np.ascontiguousarray(bmp),
        })
    return in_maps


def _finalize(results):
    total = np.float64(0.0)
    cnt = np.float64(0.0)
    for r in results:
        total += r["out"][:, 0].astype(np.float64).sum()
        cnt += r["out"][:, 1].astype(np.float64).sum()
    total = np.float32(total)
    cnt = np.float32(cnt)
    if cnt > 0.0:
        loss = total / max(cnt, np.float32(1.0))
    else:
        loss = total
    return np.float32(loss)


def run_hw(batch, beta, labels, triplets, trace=False, **kw):
    if "nc" not in _CACHE:
        _CACHE["nc"] = _build_nc()
    nc = _CACHE["nc"]
    in_maps = _prep_inputs(batch, beta, labels, triplets)
    res = run_bass_kernel_spmd(nc, in_maps, list(range(N_CORES)), trace=trace, **kw)
    return _finalize(res.results), res


def kernel(batch, beta, labels, triplets):
    loss, _ = run_hw(batch, beta, labels, triplets)
    return loss


# revision 34
# speedup vs baseline: 1.0044x; 1.0044x over previous
"""Margin-based triplet criterion (loss_fn) on 8 TRN2 NeuronCores.

Strategy (anchor-block sharding; each core owns B/8 = 512 anchor rows):
  - Host: quantize batch*0.5 to fp8 e4m3 (so the PE Gram values G/4 stay in
    e4m3 range for the fp8 store), build bT [4, 128, 4096] (D-major
    transpose, K-chunked, replicated) and aT [128, 4, 512] = the core's
    anchor-block columns (stationary). Re-sort triplets to the core owning
    their anchor (loss is a permutation-invariant sum), pad with masked
    dummies (bm=+LARGE, bp=-LARGE => zero contribution). Precompute
    per-triplet ssum = s[ia]+s[ip|n] (f32, from the dequantized rows),
    hinge offsets bm/bp, and flat gather indices into the Gram block
    laid out [p, m, col] (anchor a -> partition a%128, sub-row a//128).
  - Device: PE computes the Gram block (512 anchors x 4096) with fp8
    DoubleRow matmuls (K=256 per pass, N=512 per PSUM bank; 64 matmuls).
    DVE/ACT/Pool alternate downconverting PSUM f32 -> fp8 SBUF staging
    tiles [128, 4096] which are DMA'd (4 stores) to a DRAM scratch tile.
    One indirect (SWDGE) gather pulls the 2*NCOLS per-partition dot
    products as scalars. Epilogue: d^2 = ssum - 8*g (clamped), d =
    sqrt(d^2 + eps) on ACT, hinges pos = relu(d_ap - bm), neg =
    relu(bp - d_an), z = pos + neg, indicator z > 0, free-dim reductions
    -> [128, 2] (sum, count) per core.
  - Host: sum the 8x128 partials, loss = total / max(count, 1) if count>0.
"""

import numpy as np
import ml_dtypes
from contextlib import ExitStack

import concourse.bass as bass
import concourse.bacc as bacc
import concourse.tile as tile
from concourse import mybir
from concourse.bass_utils import run_bass_kernel_spmd

N_CORES = 8
B, D, T, C = 4096, 512, 65536, 100
A_LOC = B // N_CORES            # 512 anchors per core
KCH = 4                         # K chunks of 128 (D = 512)
M_TILES = A_LOC // 128          # 4 anchor tiles per core
NBANK = 512                     # matmul N per PSUM bank (f32)
HALF = 1024                     # columns per copy unit (2 banks)
NCOLS = 68                      # padded triplet columns per partition
NC2 = 2 * NCOLS                 # ap|an concatenated
MARGIN = 0.2
EPS = 1e-8
LARGE = 4e6

f32 = mybir.dt.float32
bf16 = mybir.dt.bfloat16
fp8 = mybir.dt.float8e4
i32 = mybir.dt.int32
FP8NP = ml_dtypes.float8_e4m3

_CACHE = {}


def _build_nc():
    nc = bacc.Bacc(
        "TRN2", target_bir_lowering=False, debug=False,
        enable_asserts=False, num_devices=N_CORES,
    )
    bT = nc.dram_tensor("bT", [KCH, 128, B], fp8, kind="ExternalInput")
    aT = nc.dram_tensor("aT", [128, KCH, A_LOC], fp8, kind="ExternalInput")
    gidx = nc.dram_tensor("gidx", [128, NC2], i32, kind="ExternalInput")
    ssum = nc.dram_tensor("ssum", [128, NC2], f32, kind="ExternalInput")
    bmp = nc.dram_tensor("bmp", [128, NC2], f32, kind="ExternalInput")
    outp = nc.dram_tensor("out", [128, 2], f32, kind="ExternalOutput")

    with tile.TileContext(nc) as tc, ExitStack() as ctx:
        const_pool = ctx.enter_context(tc.tile_pool(name="const", bufs=1))
        psum_pool = ctx.enter_context(tc.tile_pool(name="ps", bufs=3, space="PSUM"))
        gsb_pool = ctx.enter_context(tc.tile_pool(name="gsb", bufs=6))
        gdram_pool = ctx.enter_context(tc.tile_pool(name="gdram", bufs=1, space="DRAM"))
        epi_pool = ctx.enter_context(tc.tile_pool(name="epi", bufs=1))

        # Loads ordered so PE can start ASAP: stationary aT, then all 4 k
        # chunks of the first column half (enough for the h=0 units), then
        # the second half, then gather/epilogue operands.
        aT_sb = const_pool.tile([128, KCH, A_LOC], fp8)
        bT_sb = const_pool.tile([128, KCH, B], fp8)
        c0 = slice(0, B // 2)
        c1 = slice(B // 2, B)
        nc.sync.dma_start(aT_sb[:, 0:2, :], aT[:, 0:2, :])
        nc.sync.dma_start(bT_sb[:, 0, c0], bT[0, :, c0])
        nc.sync.dma_start(bT_sb[:, 1, c0], bT[1, :, c0])
        nc.sync.dma_start(aT_sb[:, 2:4, :], aT[:, 2:4, :])
        nc.sync.dma_start(bT_sb[:, 2, c0], bT[2, :, c0])
        nc.sync.dma_start(bT_sb[:, 3, c0], bT[3, :, c0])
        for k in range(KCH):
            nc.sync.dma_start(bT_sb[:, k, c1], bT[k, :, c1])

        eps_sb = const_pool.tile([128, 1], f32)
        nc.vector.memset(eps_sb[:], EPS)
        idx_sb = const_pool.tile([128, NC2], i32)
        nc.sync.dma_start(idx_sb[:], gidx[:])
        ssum_sb = const_pool.tile([128, NC2], f32)
        nc.sync.dma_start(ssum_sb[:], ssum[:])
        bmp_sb = const_pool.tile([128, NC2], f32)
        nc.sync.dma_start(bmp_sb[:], bmp[:])

        # Gram scratch, laid out [p, m, col]: anchor a = 128m + p
        g_t = gdram_pool.tile([128, M_TILES * B], fp8)

        # PE warmup: dummy matmuls gated only on the (early) aT load keep the
        # PE continuously busy until the first bT chunks land, so the real
        # matmuls issue against a ramped-up (full-rate) tensor engine.
        warm = psum_pool.tile([128, HALF // NBANK, NBANK], f32, tag="ps")
        for _ in range(6):
            nc.tensor.matmul(
                warm[:, 0, :],
                lhsT=aT_sb[:, 0:2, 0:128],
                rhs=aT_sb[:, 0:2, 0:NBANK],
                start=True, stop=True,
                perf_mode=mybir.MatmulPerfMode.DoubleRow,
            )
        for _ in range(3):
            nc.tensor.matmul(
                warm[:, 1, :],
                lhsT=aT_sb[:, 2:4, 0:128],
                rhs=aT_sb[:, 2:4, 0:NBANK],
                start=True, stop=True,
                perf_mode=mybir.MatmulPerfMode.DoubleRow,
            )

        # Gram block: units (column half h, anchor tile m); h-outer so the
        # first 4 units only need the first half of the bT columns.
        # Both copy engines split each unit (halves PSUM release latency).
        for h in range(B // HALF):
            for m in range(M_TILES):
                ps = psum_pool.tile([128, HALF // NBANK, NBANK], f32, tag="ps")
                # k2-outer so PE only waits on bT chunks 2*k2, 2*k2+1
                for k2 in range(KCH // 2):
                    for b in range(HALF // NBANK):
                        nc.tensor.matmul(
                            ps[:, b, :],
                            lhsT=aT_sb[:, 2 * k2:2 * k2 + 2,
                                       m * 128:(m + 1) * 128],
                            rhs=bT_sb[:, 2 * k2:2 * k2 + 2,
                                      h * HALF + b * NBANK:
                                      h * HALF + (b + 1) * NBANK],
                            start=(k2 == 0), stop=(k2 == KCH // 2 - 1),
                            perf_mode=mybir.MatmulPerfMode.DoubleRow,
                        )
                gsb = gsb_pool.tile([128, HALF], fp8, tag="gsb")
                if (h * M_TILES + m) % 2 == 1:
                    nc.scalar.copy(out=gsb[:], in_=ps[:, :, :])
                else:
                    nc.vector.tensor_copy(out=gsb[:], in_=ps[:, :, :])
                nc.sync.dma_start(
                    g_t[:, m * B + h * HALF:m * B + (h + 1) * HALF], gsb[:])

        # Per-triplet dot products: scalar gather from the DRAM Gram block
        g_vals = epi_pool.tile([128, NC2], fp8, tag="gv")
        nc.gpsimd.indirect_dma_start(
            out=g_vals[:],
            out_offset=None,
            in_=g_t[:],
            in_offset=bass.IndirectOffsetOnAxis(ap=idx_sb[:], axis=1),
        )

        # d^2 = ssum - 8*g = -8*(g - ssum/8); host passes ssum/8.
        dif = epi_pool.tile([128, NC2], f32, tag="dif")
        nc.vector.tensor_tensor(
            out=dif[:], in0=g_vals[:], in1=ssum_sb[:],
            op=mybir.AluOpType.subtract)
        dsq = epi_pool.tile([128, NC2], f32, tag="dsq")
        nc.vector.tensor_scalar(
            out=dsq[:], in0=dif[:], scalar1=-8.0, scalar2=0.0,
            op0=mybir.AluOpType.mult, op1=mybir.AluOpType.max)
        dist = epi_pool.tile([128, NC2], f32, tag="dist")
        nc.scalar.activation(
            out=dist[:], in_=dsq[:],
            func=mybir.ActivationFunctionType.Sqrt, bias=eps_sb[:])

        # hinges: pos = relu(d_ap - bm) (DVE), neg = relu(bp - d_an) (Pool)
        hing = epi_pool.tile([128, NC2], f32, tag="hing")
        nc.vector.tensor_tensor(
            out=hing[:, 0:NCOLS], in0=dist[:, 0:NCOLS], in1=bmp_sb[:, 0:NCOLS],
            op=mybir.AluOpType.subtract)
        nc.vector.tensor_tensor(
            out=hing[:, NCOLS:NC2], in0=bmp_sb[:, NCOLS:NC2],
            in1=dist[:, NCOLS:NC2], op=mybir.AluOpType.subtract)
        nc.vector.tensor_scalar_max(hing[:], hing[:], 0.0)
        z = epi_pool.tile([128, NCOLS], f32, tag="z")
        outsb = epi_pool.tile([128, 2], f32, tag="outsb")
        nc.vector.scalar_tensor_tensor(
            out=z[:], in0=hing[:, 0:NCOLS], scalar=0.0,
            in1=hing[:, NCOLS:NC2], op0=mybir.AluOpType.add,
            op1=mybir.AluOpType.add, accum_out=outsb[:, 0:1])
        ind = epi_pool.tile([128, NCOLS], f32, tag="ind")
        nc.vector.tensor_scalar(
            out=ind[:], in0=z[:], scalar1=0.0, scalar2=0.0,
            op0=mybir.AluOpType.is_gt, op1=mybir.AluOpType.add,
            accum_out=outsb[:, 1:2])
        nc.sync.dma_start(outp[:], outsb[:])

    nc.compile()
    return nc


def _prep_inputs(batch, beta, labels, triplets):
    batch = np.asarray(batch, dtype=np.float32)
    beta = np.asarray(beta, dtype=np.float32)
    labels = np.asarray(labels).astype(np.int64)
    triplets = np.asarray(triplets).astype(np.int64)

    q = (0.5 * batch).astype(FP8NP)          # device rows (scaled by 1/2)
    qf = q.astype(np.float32)
    # effective embedding is 2*q; s = |2q|^2
    s = 4.0 * (qf.astype(np.float64) ** 2).sum(axis=1)
    s = s.astype(np.float32)

    # bT[k, d, j] = q[j, 128k + d]
    bT_all = np.ascontiguousarray(q.T.reshape(KCH, 128, B))

    ia, ip, iN = triplets[:, 0], triplets[:, 1], triplets[:, 2]
    core = ia // A_LOC
    al = ia % A_LOC
    b = beta[labels[ia]].astype(np.float32)

    # Gram scratch layout [p, m, col]: flat = (al%128)*(M_TILES*B) + (al//128)*B + col
    CAP = 128 * NCOLS
    in_maps = []
    for c in range(N_CORES):
        sel = np.nonzero(core == c)[0]
        n = len(sel)
        assert n <= CAP, f"core {c} overflow: {n} > {CAP}"

        def packi(vals, fill):
            arr = np.full(CAP, fill, dtype=np.int64)
            arr[:n] = vals
            return arr.reshape(NCOLS, 128).T.astype(np.int32)

        def packf(vals, fill):
            arr = np.full(CAP, fill, dtype=np.float32)
            arr[:n] = vals
            return arr.reshape(NCOLS, 128).T

        alc = al[sel]
        base = (alc % 128) * (M_TILES * B) + (alc // 128) * B
        gidx = np.concatenate(
            [packi(base + ip[sel], 0), packi(base + iN[sel], 0)], axis=1)
        # device computes d^2 = -8*(g - ssum/8): pass ssum/8
        ssum = np.concatenate(
            [packf((s[ia[sel]] + s[ip[sel]]) / 8.0, 0.0),
             packf((s[ia[sel]] + s[iN[sel]]) / 8.0, 0.0)], axis=1)
        # first half bm (pad +LARGE kills pos), second half bp (pad -LARGE
        # kills neg: device computes relu(bp - d))
        bmp = np.concatenate(
            [packf(b[sel] - MARGIN, LARGE), packf(b[sel] + MARGIN, -LARGE)],
            axis=1)

        aTc = np.ascontiguousarray(
            bT_all[:, :, c * A_LOC:(c + 1) * A_LOC].transpose(1, 0, 2))

        in_maps.append({
            "bT": bT_all,
            "aT": aTc,
            "gidx": np.ascontiguousarray(gidx),
            "ssum": np.ascontiguousarray(ssum),
            "bmp": np.ascontiguousarray(bmp),
        })
    return in_maps


def _finalize(results):
    total = np.float64(0.0)
    cnt = np.float64(0.0)
    for r in results:
        total += r["out"][:, 0].astype(np.float64).sum()
        cnt += r["out"][:, 1].astype(np.float64).sum()
    total = np.float32(total)
    cnt = np.float32(cnt)
    if cnt > 0.0:
        loss = total / max(cnt, np.float32(1.0))
    else:
        loss = total
    return np.float32(loss)


def run_hw(batch, beta, labels, triplets, trace=False, **kw):
    if "nc" not in _CACHE:
        _CACHE["nc"] = _build_nc()
    nc = _CACHE["nc"]
    in_maps = _prep_inputs(batch, beta, labels, triplets)
    res = run_bass_kernel_spmd(nc, in_maps, list(range(N_CORES)), trace=trace, **kw)
    return _finalize(res.results), res


def kernel(batch, beta, labels, triplets):
    loss, _ = run_hw(batch, beta, labels, triplets)
    return loss
